# revision 25
# baseline (speedup 1.0000x reference)
"""GRU cell kernel for Trainium2, data-parallel across 8 NeuronCores.

Per core: batch shard of 1024 rows; weights replicated.
  u  = sigmoid(x @ Wxu + h @ Whu + bu)
  r  = sigmoid(x @ Wxr + h @ Whr + br)
  c' = tanh  (x @ Wxc + (h*r) @ Whc + bc)
  c  = u*c' + (1-u)*h

All layout work happens on the host (free - the harness times only the
NEFF): x and h are transposed to [feature, batch] and converted to bf16
before upload, weights are uploaded as bf16 in natural layout, and the
output comes back transposed [H, batch] fp32 and is untransposed on the
host.  On-chip the kernel is a pure matmul stream: 768 bf16 matmuls
(stationary = 128-col weight slice, moving = 512-col activation slice)
plus scalar-engine activations and a vector-engine blend.
"""

import os
import sys

import numpy as np

B = 8192
E = 1024
H = 1024
NCORES = 8
B_SH = B // NCORES  # 1024 rows per core

P = 128
KE = E // P   # 8 contraction chunks
NJ = H // P   # 8 output feature chunks
BN = 512      # moving free-dim per matmul (one PSUM bank of fp32)
NB = B_SH // BN  # 2

W_NAMES = ("Wxu", "Whu", "Wxr", "Whr", "Wxc", "Whc")
B_NAMES = ("bu", "br", "bc")

_NC_CACHE = {}


def _ensure_paths():
    for p in ("/opt/trn_rl_repo", "/root/.axon_site/_ro/trn_rl_repo"):
        if os.path.isdir(p) and p not in sys.path:
            sys.path.insert(0, p)


def _build_nc():
    import concourse.bass as bass
    import concourse.mybir as mybir
    from concourse.tile import TileContext

    f32 = mybir.dt.float32
    bf16 = mybir.dt.bfloat16
    AF = mybir.ActivationFunctionType

    nc = bass.Bass()
    x_d = nc.dram_tensor("xT", [E, B_SH], bf16, kind="ExternalInput")
    h_d = nc.dram_tensor("hT", [H, B_SH], bf16, kind="ExternalInput")
    w_d = {n: nc.dram_tensor(n, [E, H], bf16, kind="ExternalInput") for n in W_NAMES}
    b_d = nc.dram_tensor("btab", [P, 3 * NJ], f32, kind="ExternalInput")
    out_d = nc.dram_tensor("out", [H, B_SH], f32, kind="ExternalOutput")

    with TileContext(nc) as tc:
        with (
            tc.tile_pool(name="sb", bufs=1) as sb,
            tc.tile_pool(name="psum", bufs=1, space="PSUM") as pp,
        ):
            xt = [sb.tile([P, B_SH], bf16, tag=f"xt{k}", name=f"xt{k}", bufs=1) for k in range(KE)]
            ht = [sb.tile([P, B_SH], bf16, tag=f"ht{k}", name=f"ht{k}", bufs=1) for k in range(KE)]
            wt = {
                n: [sb.tile([P, H], bf16, tag=f"w_{n}_{k}", name=f"w_{n}_{k}", bufs=1) for k in range(KE)]
                for n in W_NAMES
            }
            ut = [sb.tile([P, B_SH], f32, tag=f"ut{j}", name=f"ut{j}", bufs=1) for j in range(NJ)]
            rh = [sb.tile([P, B_SH], bf16, tag=f"rh{j}", name=f"rh{j}", bufs=1) for j in range(NJ)]
            # uh1m = (u-1)*h, precomputed off the critical path: the final
            # blend is then c = c'*u - uh1m (two vector ops at the tail).
            uh1m = [sb.tile([P, B_SH], bf16, tag=f"uh{j}", name=f"uh{j}", bufs=1) for j in range(NJ)]
            bias = sb.tile([P, 3 * NJ], f32, tag="bias", bufs=1)

            # ---- PE warm-up: a few dummy matmuls so the HAM clock gate
            # starts warming while the first DMAs are still in flight ----
            warm = sb.tile([P, BN], bf16, tag="warm", bufs=1)
            nc.vector.memset(warm[:], 0.0)
            wps = pp.tile([P, BN], f32, tag="mm", name="warmps", bufs=8)
            for _ in range(3):
                nc.tensor.matmul(
                    wps[:], warm[:, 0:P], warm[:], start=True, stop=True
                )

            # ---- all input DMAs on the sync ring, strictly in consumption
            # order: the single queue then acts as an implicit priority
            # scheduler (splitting across rings starves the r-gate tiles,
            # which need most of the HBM bandwidth early) ----
            nc.scalar.dma_start(bias[:], b_d[:, :])
            # k=0 tiles arrive as halves so the first matmul's operands land
            # ~1us sooner (the first chain needs only 512 cols of each)
            nc.sync.dma_start(xt[0][:, 0:BN], x_d[0:P, 0:BN])
            nc.sync.dma_start(wt["Wxr"][0][:, 0 : 4 * P], w_d["Wxr"][0:P, 0 : 4 * P])
            nc.sync.dma_start(xt[0][:, BN:B_SH], x_d[0:P, BN:B_SH])
            for k in range(1, KE):
                nc.sync.dma_start(xt[k][:], x_d[k * P : (k + 1) * P, :])
                nc.sync.dma_start(wt["Wxr"][k][:], w_d["Wxr"][k * P : (k + 1) * P, :])
            nc.sync.dma_start(
                wt["Wxr"][0][:, 4 * P : H], w_d["Wxr"][0:P, 4 * P : H]
            )
            for k in range(KE):
                nc.sync.dma_start(ht[k][:], h_d[k * P : (k + 1) * P, :])
                nc.sync.dma_start(wt["Whr"][k][:], w_d["Whr"][k * P : (k + 1) * P, :])
            for k in range(KE):
                nc.sync.dma_start(wt["Wxu"][k][:], w_d["Wxu"][k * P : (k + 1) * P, :])
                nc.sync.dma_start(wt["Whu"][k][:], w_d["Whu"][k * P : (k + 1) * P, :])
            for k in range(KE):
                nc.sync.dma_start(wt["Wxc"][k][:], w_d["Wxc"][k * P : (k + 1) * P, :])
                nc.sync.dma_start(wt["Whc"][k][:], w_d["Whc"][k * P : (k + 1) * P, :])

            def gate_psums(wx, wh, hside, j):
                """16-matmul accumulation chains for output chunk j, both
                batch halves interleaved so consecutive matmuls share the
                stationary operand."""
                jsl = slice(j * P, (j + 1) * P)
                ps = [pp.tile([P, BN], f32, tag="mm", name=f"ps{j}_{_n}", bufs=8) for _n in range(NB)]
                for k in range(KE):
                    for n in range(NB):
                        nc.tensor.matmul(
                            ps[n][:],
                            wt[wx][k][:, jsl],
                            xt[k][:, n * BN : (n + 1) * BN],
                            start=(k == 0),
                            stop=False,
                        )
                for k in range(KE):
                    for n in range(NB):
                        nc.tensor.matmul(
                            ps[n][:],
                            wt[wh][k][:, jsl],
                            hside[k][:, n * BN : (n + 1) * BN],
                            start=False,
                            stop=(k == KE - 1),
                        )
                return ps

            # ---- gate r: sigmoid -> multiply by h (kept transposed) ----
            # k-outer over groups of 4 j's (8 PSUM banks) so the matmul
            # stream keeps pace with the weight/activation DMAs still in
            # flight at kernel start instead of stalling inside one chain.
            for jlo in range(0, NJ, 4):
                grp = range(jlo, jlo + 4)
                gps = {
                    j: [
                        pp.tile([P, BN], f32, tag="mm", name=f"psr{j}_{_n}", bufs=8)
                        for _n in range(NB)
                    ]
                    for j in grp
                }
                for k in range(KE):
                    for j in grp:
                        for n in range(NB):
                            nc.tensor.matmul(
                                gps[j][n][:],
                                wt["Wxr"][k][:, j * P : (j + 1) * P],
                                xt[k][:, n * BN : (n + 1) * BN],
                                start=(k == 0),
                                stop=False,
                            )
                for k in range(KE):
                    for j in grp:
                        for n in range(NB):
                            nc.tensor.matmul(
                                gps[j][n][:],
                                wt["Whr"][k][:, j * P : (j + 1) * P],
                                ht[k][:, n * BN : (n + 1) * BN],
                                start=False,
                                stop=(k == KE - 1),
                            )
                for j in grp:
                    for n in range(NB):
                        sl = slice(n * BN, (n + 1) * BN)
                        nc.scalar.activation(
                            rh[j][:, sl], gps[j][n][:], AF.Sigmoid,
                            bias=bias[:, j : j + 1],
                        )
                    nc.vector.tensor_mul(rh[j][:], rh[j][:], ht[j][:])

            # ---- gate u: sigmoid fp32; also precompute (u-1)*h ----
            for j in range(NJ):
                ps = gate_psums("Wxu", "Whu", ht, j)
                for n in range(NB):
                    sl = slice(n * BN, (n + 1) * BN)
                    nc.scalar.activation(
                        ut[j][:, sl], ps[n][:], AF.Sigmoid,
                        bias=bias[:, NJ + j : NJ + j + 1],
                    )
                nc.vector.scalar_tensor_tensor(
                    uh1m[j][:], ut[j][:], 1.0, ht[j][:],
                    op0=mybir.AluOpType.subtract, op1=mybir.AluOpType.mult,
                )

            # ---- candidate + blend + store, pipelined per batch half.
            # n-outer: each half's 16-matmul chain finishes ~3.5us before
            # the next, so activation+blend+store overlap the matmuls ----
            for j in range(NJ):
                cc = sb.tile([P, B_SH], f32, tag="cc", bufs=3)
                for n in range(NB):
                    sl = slice(n * BN, (n + 1) * BN)
                    ps = pp.tile([P, BN], f32, tag="mm", name=f"psc{j}_{n}", bufs=8)
                    for k in range(KE):
                        nc.tensor.matmul(
                            ps[:],
                            wt["Wxc"][k][:, j * P : (j + 1) * P],
                            xt[k][:, sl],
                            start=(k == 0),
                            stop=False,
                        )
                    for k in range(KE):
                        nc.tensor.matmul(
                            ps[:],
                            wt["Whc"][k][:, j * P : (j + 1) * P],
                            rh[k][:, sl],
                            start=False,
                            stop=(k == KE - 1),
                        )
                    # the very last half drains in two 256-wide pieces so
                    # the final act->blend->store chain is half as deep
                    if j == NJ - 1 and n == NB - 1:
                        bounds = (0, BN // 2, BN)
                    else:
                        bounds = (0, BN)
                    for q in range(len(bounds) - 1):
                        qsl = slice(n * BN + bounds[q], n * BN + bounds[q + 1])
                        psl = slice(bounds[q], bounds[q + 1])
                        nc.scalar.activation(
                            cc[:, qsl], ps[:, psl], AF.Tanh,
                            bias=bias[:, 2 * NJ + j : 2 * NJ + j + 1],
                        )
                        # c = c'*u - (u-1)*h
                        nc.vector.tensor_mul(cc[:, qsl], cc[:, qsl], ut[j][:, qsl])
                        nc.vector.tensor_sub(cc[:, qsl], cc[:, qsl], uh1m[j][:, qsl])
                        nc.sync.dma_start(
                            out_d[j * P : (j + 1) * P, qsl],
                            cc[:, qsl],
                        )

    _split_matmul_waits(nc, mybir)
    return nc


def _split_matmul_waits(nc, mybir):
    """Walrus codegen allows only one sync-wait on a Matmult (it lowers to an
    LDW+MM pair).  Spill extra waits onto a PE NoOp placed just before."""
    n_fixed = 0
    blocks = list(nc.m.functions[0].blocks)
    origs = [list(b.instructions) for b in blocks]
    spill_nops = {}  # id(inst) -> [nop insts]
    for orig in origs:
        for inst in orig:
            si = inst.sync_info
            if (
                si is not None
                and si.on_wait
                and len(si.on_wait) > 1
            ):
                waits = list(si.on_wait)
                eng = nc.engines[inst.engine]
                nops = []
                for w in waits[:-1]:
                    nop = eng.nop(hint="waitspill").ins
                    nop.sync_info = mybir.SyncInfo(on_wait=[w], on_update=[])
                    nops.append(nop)
                inst.sync_info = mybir.SyncInfo(
                    on_wait=waits[-1:], on_update=list(si.on_update or [])
                )
                spill_nops[id(inst)] = nops
                n_fixed += 1
    for blk, orig in zip(blocks, origs):
        new_list = []
        for inst in orig:
            if id(inst) in spill_nops:
                new_list.extend(spill_nops[id(inst)])
            new_list.append(inst)
        # rebuilding from `orig` also drops any freshly created nops that
        # bass appended to this block's tail
        blk.instructions[:] = new_list
    return n_fixed


def get_nc():
    if "nc" not in _NC_CACHE:
        _ensure_paths()
        _NC_CACHE["nc"] = _build_nc()
    return _NC_CACHE["nc"]


def build_in_maps(inputs):
    """Host-side prep: transpose x/h, convert to bf16, pack biases."""
    import ml_dtypes

    bf = ml_dtypes.bfloat16
    x = np.asarray(inputs["input"], dtype=np.float32)
    h = np.asarray(inputs["hidden_state"], dtype=np.float32)
    xT = np.ascontiguousarray(x.astype(bf).T)  # [E, B]
    hT = np.ascontiguousarray(h.astype(bf).T)  # [H, B]
    shared = {
        n: np.ascontiguousarray(np.asarray(inputs[n], dtype=np.float32).astype(bf))
        for n in W_NAMES
    }
    btab = np.zeros((P, 3 * NJ), np.float32)
    for g, nm in enumerate(("br", "bu", "bc")):
        b = np.asarray(inputs[nm], dtype=np.float32).reshape(H)
        btab[:, g * NJ : (g + 1) * NJ] = b.reshape(NJ, P).T
    shared["btab"] = btab

    in_maps = []
    for c in range(NCORES):
        m = {
            "xT": np.ascontiguousarray(xT[:, c * B_SH : (c + 1) * B_SH]),
            "hT": np.ascontiguousarray(hT[:, c * B_SH : (c + 1) * B_SH]),
        }
        m.update(shared)
        in_maps.append(m)
    return in_maps


def assemble_output(res):
    outT = np.concatenate(
        [np.asarray(res.results[c]["out"]) for c in range(NCORES)], axis=1
    )  # [H, B]
    return np.ascontiguousarray(outT.T).astype(np.float32)


def kernel(**inputs):
    _ensure_paths()
    from concourse.bass_utils import run_bass_kernel_spmd

    nc = get_nc()
    in_maps = build_in_maps(inputs)
    res = run_bass_kernel_spmd(nc, in_maps, list(range(NCORES)))
    return assemble_output(res)


# revision 27
# speedup vs baseline: 1.0060x; 1.0060x over previous
"""GRU cell kernel for Trainium2, data-parallel across 8 NeuronCores.

Per core: batch shard of 1024 rows; weights replicated.
  u  = sigmoid(x @ Wxu + h @ Whu + bu)
  r  = sigmoid(x @ Wxr + h @ Whr + br)
  c' = tanh  (x @ Wxc + (h*r) @ Whc + bc)
  c  = u*c' + (1-u)*h

All layout work happens on the host (free - the harness times only the
NEFF): x and h are transposed to [feature, batch] and converted to bf16
before upload, weights are uploaded as bf16 in natural layout, and the
output comes back transposed [H, batch] fp32 and is untransposed on the
host.  On-chip the kernel is a pure matmul stream: 768 bf16 matmuls
(stationary = 128-col weight slice, moving = 512-col activation slice)
plus scalar-engine activations and a vector-engine blend.
"""

import os
import sys

import numpy as np

B = 8192
E = 1024
H = 1024
NCORES = 8
B_SH = B // NCORES  # 1024 rows per core

P = 128
KE = E // P   # 8 contraction chunks
NJ = H // P   # 8 output feature chunks
BN = 512      # moving free-dim per matmul (one PSUM bank of fp32)
NB = B_SH // BN  # 2

W_NAMES = ("Wxu", "Whu", "Wxr", "Whr", "Wxc", "Whc")
B_NAMES = ("bu", "br", "bc")

_NC_CACHE = {}


def _ensure_paths():
    for p in ("/opt/trn_rl_repo", "/root/.axon_site/_ro/trn_rl_repo"):
        if os.path.isdir(p) and p not in sys.path:
            sys.path.insert(0, p)


def _build_nc():
    import concourse.bass as bass
    import concourse.mybir as mybir
    from concourse.tile import TileContext

    f32 = mybir.dt.float32
    bf16 = mybir.dt.bfloat16
    AF = mybir.ActivationFunctionType

    nc = bass.Bass()
    x_d = nc.dram_tensor("xT", [E, B_SH], bf16, kind="ExternalInput")
    h_d = nc.dram_tensor("hT", [H, B_SH], bf16, kind="ExternalInput")
    w_d = {n: nc.dram_tensor(n, [E, H], bf16, kind="ExternalInput") for n in W_NAMES}
    b_d = nc.dram_tensor("btab", [P, 3 * NJ], f32, kind="ExternalInput")
    out_d = nc.dram_tensor("out", [H, B_SH], f32, kind="ExternalOutput")

    with TileContext(nc) as tc:
        with (
            tc.tile_pool(name="sb", bufs=1) as sb,
            tc.tile_pool(name="psum", bufs=1, space="PSUM") as pp,
        ):
            xt = [sb.tile([P, B_SH], bf16, tag=f"xt{k}", name=f"xt{k}", bufs=1) for k in range(KE)]
            ht = [sb.tile([P, B_SH], bf16, tag=f"ht{k}", name=f"ht{k}", bufs=1) for k in range(KE)]
            wt = {
                n: [sb.tile([P, H], bf16, tag=f"w_{n}_{k}", name=f"w_{n}_{k}", bufs=1) for k in range(KE)]
                for n in W_NAMES
            }
            ut = [sb.tile([P, B_SH], f32, tag=f"ut{j}", name=f"ut{j}", bufs=1) for j in range(NJ)]
            rh = [sb.tile([P, B_SH], bf16, tag=f"rh{j}", name=f"rh{j}", bufs=1) for j in range(NJ)]
            # uh1m = (u-1)*h, precomputed off the critical path: the final
            # blend is then c = c'*u - uh1m (two vector ops at the tail).
            uh1m = [sb.tile([P, B_SH], bf16, tag=f"uh{j}", name=f"uh{j}", bufs=1) for j in range(NJ)]
            bias = sb.tile([P, 3 * NJ], f32, tag="bias", bufs=1)

            # ---- PE warm-up: a few dummy matmuls so the HAM clock gate
            # starts warming while the first DMAs are still in flight ----
            warm = sb.tile([P, BN], bf16, tag="warm", bufs=1)
            nc.vector.memset(warm[:], 0.0)
            wps = pp.tile([P, BN], f32, tag="mm", name="warmps", bufs=8)
            for _ in range(3):
                nc.tensor.matmul(
                    wps[:], warm[:, 0:P], warm[:], start=True, stop=True
                )

            # ---- all input DMAs on the sync ring, strictly in consumption
            # order: the single queue then acts as an implicit priority
            # scheduler (splitting across rings starves the r-gate tiles,
            # which need most of the HBM bandwidth early) ----
            # k=0 tiles arrive as halves, and the first x- and weight-tiles
            # go down two different rings, so the first matmul's operands
            # (and their completion receipts) land in parallel ~1.3us sooner
            nc.scalar.dma_start(wt["Wxr"][0][:, 0 : 4 * P], w_d["Wxr"][0:P, 0 : 4 * P])
            nc.scalar.dma_start(bias[:], b_d[:, :])
            nc.sync.dma_start(xt[0][:, 0:BN], x_d[0:P, 0:BN])
            nc.sync.dma_start(xt[0][:, BN:B_SH], x_d[0:P, BN:B_SH])
            for k in range(1, KE):
                nc.sync.dma_start(xt[k][:], x_d[k * P : (k + 1) * P, :])
                nc.sync.dma_start(wt["Wxr"][k][:], w_d["Wxr"][k * P : (k + 1) * P, :])
            nc.sync.dma_start(
                wt["Wxr"][0][:, 4 * P : H], w_d["Wxr"][0:P, 4 * P : H]
            )
            for k in range(KE):
                nc.sync.dma_start(ht[k][:], h_d[k * P : (k + 1) * P, :])
                nc.sync.dma_start(wt["Whr"][k][:], w_d["Whr"][k * P : (k + 1) * P, :])
            for k in range(KE):
                nc.sync.dma_start(wt["Wxu"][k][:], w_d["Wxu"][k * P : (k + 1) * P, :])
                nc.sync.dma_start(wt["Whu"][k][:], w_d["Whu"][k * P : (k + 1) * P, :])
            for k in range(KE):
                nc.sync.dma_start(wt["Wxc"][k][:], w_d["Wxc"][k * P : (k + 1) * P, :])
                nc.sync.dma_start(wt["Whc"][k][:], w_d["Whc"][k * P : (k + 1) * P, :])

            def gate_psums(wx, wh, hside, j):
                """16-matmul accumulation chains for output chunk j, both
                batch halves interleaved so consecutive matmuls share the
                stationary operand."""
                jsl = slice(j * P, (j + 1) * P)
                ps = [pp.tile([P, BN], f32, tag="mm", name=f"ps{j}_{_n}", bufs=8) for _n in range(NB)]
                for k in range(KE):
                    for n in range(NB):
                        nc.tensor.matmul(
                            ps[n][:],
                            wt[wx][k][:, jsl],
                            xt[k][:, n * BN : (n + 1) * BN],
                            start=(k == 0),
                            stop=False,
                        )
                for k in range(KE):
                    for n in range(NB):
                        nc.tensor.matmul(
                            ps[n][:],
                            wt[wh][k][:, jsl],
                            hside[k][:, n * BN : (n + 1) * BN],
                            start=False,
                            stop=(k == KE - 1),
                        )
                return ps

            # ---- gate r: sigmoid -> multiply by h (kept transposed) ----
            # k-outer over groups of 4 j's (8 PSUM banks) so the matmul
            # stream keeps pace with the weight/activation DMAs still in
            # flight at kernel start instead of stalling inside one chain.
            for jlo in range(0, NJ, 4):
                grp = range(jlo, jlo + 4)
                gps = {
                    j: [
                        pp.tile([P, BN], f32, tag="mm", name=f"psr{j}_{_n}", bufs=8)
                        for _n in range(NB)
                    ]
                    for j in grp
                }
                for k in range(KE):
                    for j in grp:
                        for n in range(NB):
                            nc.tensor.matmul(
                                gps[j][n][:],
                                wt["Wxr"][k][:, j * P : (j + 1) * P],
                                xt[k][:, n * BN : (n + 1) * BN],
                                start=(k == 0),
                                stop=False,
                            )
                for k in range(KE):
                    for j in grp:
                        for n in range(NB):
                            nc.tensor.matmul(
                                gps[j][n][:],
                                wt["Whr"][k][:, j * P : (j + 1) * P],
                                ht[k][:, n * BN : (n + 1) * BN],
                                start=False,
                                stop=(k == KE - 1),
                            )
                for j in grp:
                    for n in range(NB):
                        sl = slice(n * BN, (n + 1) * BN)
                        nc.scalar.activation(
                            rh[j][:, sl], gps[j][n][:], AF.Sigmoid,
                            bias=bias[:, j : j + 1],
                        )
                    nc.vector.tensor_mul(rh[j][:], rh[j][:], ht[j][:])

            # ---- gate u: sigmoid fp32; also precompute (u-1)*h ----
            for j in range(NJ):
                ps = gate_psums("Wxu", "Whu", ht, j)
                for n in range(NB):
                    sl = slice(n * BN, (n + 1) * BN)
                    nc.scalar.activation(
                        ut[j][:, sl], ps[n][:], AF.Sigmoid,
                        bias=bias[:, NJ + j : NJ + j + 1],
                    )
                nc.vector.scalar_tensor_tensor(
                    uh1m[j][:], ut[j][:], 1.0, ht[j][:],
                    op0=mybir.AluOpType.subtract, op1=mybir.AluOpType.mult,
                )

            # ---- candidate + blend + store, pipelined per batch half.
            # n-outer: each half's 16-matmul chain finishes ~3.5us before
            # the next, so activation+blend+store overlap the matmuls ----
            for j in range(NJ):
                cc = sb.tile([P, B_SH], f32, tag="cc", bufs=3)
                for n in range(NB):
                    sl = slice(n * BN, (n + 1) * BN)
                    ps = pp.tile([P, BN], f32, tag="mm", name=f"psc{j}_{n}", bufs=8)
                    for k in range(KE):
                        nc.tensor.matmul(
                            ps[:],
                            wt["Wxc"][k][:, j * P : (j + 1) * P],
                            xt[k][:, sl],
                            start=(k == 0),
                            stop=False,
                        )
                    for k in range(KE):
                        nc.tensor.matmul(
                            ps[:],
                            wt["Whc"][k][:, j * P : (j + 1) * P],
                            rh[k][:, sl],
                            start=False,
                            stop=(k == KE - 1),
                        )
                    # the very last half drains in a 448- then a 64-wide
                    # piece (the tiny final piece's store goes down the
                    # scalar ring) so the terminal act->blend->store->receipt
                    # chain is as short as possible
                    last = j == NJ - 1 and n == NB - 1
                    bounds = (0, BN - 64, BN) if last else (0, BN)
                    for q in range(len(bounds) - 1):
                        qsl = slice(n * BN + bounds[q], n * BN + bounds[q + 1])
                        psl = slice(bounds[q], bounds[q + 1])
                        nc.scalar.activation(
                            cc[:, qsl], ps[:, psl], AF.Tanh,
                            bias=bias[:, 2 * NJ + j : 2 * NJ + j + 1],
                        )
                        # c = c'*u - (u-1)*h
                        nc.vector.tensor_mul(cc[:, qsl], cc[:, qsl], ut[j][:, qsl])
                        nc.vector.tensor_sub(cc[:, qsl], cc[:, qsl], uh1m[j][:, qsl])
                        eng = nc.scalar if (last and q == 1) else nc.sync
                        eng.dma_start(
                            out_d[j * P : (j + 1) * P, qsl],
                            cc[:, qsl],
                        )

    _split_matmul_waits(nc, mybir)
    return nc


def _split_matmul_waits(nc, mybir):
    """Walrus codegen allows only one sync-wait on a Matmult (it lowers to an
    LDW+MM pair).  Spill extra waits onto a PE NoOp placed just before."""
    n_fixed = 0
    blocks = list(nc.m.functions[0].blocks)
    origs = [list(b.instructions) for b in blocks]
    spill_nops = {}  # id(inst) -> [nop insts]
    for orig in origs:
        for inst in orig:
            si = inst.sync_info
            if (
                si is not None
                and si.on_wait
                and len(si.on_wait) > 1
            ):
                waits = list(si.on_wait)
                eng = nc.engines[inst.engine]
                nops = []
                for w in waits[:-1]:
                    nop = eng.nop(hint="waitspill").ins
                    nop.sync_info = mybir.SyncInfo(on_wait=[w], on_update=[])
                    nops.append(nop)
                inst.sync_info = mybir.SyncInfo(
                    on_wait=waits[-1:], on_update=list(si.on_update or [])
                )
                spill_nops[id(inst)] = nops
                n_fixed += 1
    for blk, orig in zip(blocks, origs):
        new_list = []
        for inst in orig:
            if id(inst) in spill_nops:
                new_list.extend(spill_nops[id(inst)])
            new_list.append(inst)
        # rebuilding from `orig` also drops any freshly created nops that
        # bass appended to this block's tail
        blk.instructions[:] = new_list
    return n_fixed


def get_nc():
    if "nc" not in _NC_CACHE:
        _ensure_paths()
        _NC_CACHE["nc"] = _build_nc()
    return _NC_CACHE["nc"]


def build_in_maps(inputs):
    """Host-side prep: transpose x/h, convert to bf16, pack biases."""
    import ml_dtypes

    bf = ml_dtypes.bfloat16
    x = np.asarray(inputs["input"], dtype=np.float32)
    h = np.asarray(inputs["hidden_state"], dtype=np.float32)
    xT = np.ascontiguousarray(x.astype(bf).T)  # [E, B]
    hT = np.ascontiguousarray(h.astype(bf).T)  # [H, B]
    shared = {
        n: np.ascontiguousarray(np.asarray(inputs[n], dtype=np.float32).astype(bf))
        for n in W_NAMES
    }
    btab = np.zeros((P, 3 * NJ), np.float32)
    for g, nm in enumerate(("br", "bu", "bc")):
        b = np.asarray(inputs[nm], dtype=np.float32).reshape(H)
        btab[:, g * NJ : (g + 1) * NJ] = b.reshape(NJ, P).T
    shared["btab"] = btab

    in_maps = []
    for c in range(NCORES):
        m = {
            "xT": np.ascontiguousarray(xT[:, c * B_SH : (c + 1) * B_SH]),
            "hT": np.ascontiguousarray(hT[:, c * B_SH : (c + 1) * B_SH]),
        }
        m.update(shared)
        in_maps.append(m)
    return in_maps


def assemble_output(res):
    outT = np.concatenate(
        [np.asarray(res.results[c]["out"]) for c in range(NCORES)], axis=1
    )  # [H, B]
    return np.ascontiguousarray(outT.T).astype(np.float32)


def kernel(**inputs):
    _ensure_paths()
    from concourse.bass_utils import run_bass_kernel_spmd

    nc = get_nc()
    in_maps = build_in_maps(inputs)
    res = run_bass_kernel_spmd(nc, in_maps, list(range(NCORES)))
    return assemble_output(res)
